# revision 1
# baseline (speedup 1.0000x reference)
"""GIN message-passing GNN on 8 Trainium2 NeuronCores (Bass/Tile).

Strategy (self-contained; shapes hardcoded for the 100k-node / 1.6M-edge /
128-dim / 10-layer / 64-graph problem):

- Nodes are partitioned into 8 contiguous ranges of 12500; each core owns the
  edges whose *destination* falls in its range.
- Each core keeps a full replica of the node features in its HBM. Per layer it
  gathers x[src] for its ~200k edges with one indirect DMA per 128-dst tile
  (edges pre-sorted by dst tile host-side, padded to a multiple of 128 with
  src=0 / dstoff=-1; pad length is the max over cores so the program is SPMD).
- The segment-sum (scatter-add) becomes a matmul: for each 128-edge chunk,
  PSUM[feat, dst] += contract_edges(gathered[edge, feat], onehot[edge, dst]),
  with the one-hot built on the vector engine by a broadcast is_equal against
  a resident iota row. Padding lanes have dstoff=-1 -> all-zero column.
- The GIN MLP runs in the transposed [feat, node] orientation so both matmuls
  chain without transposes; the per-core x^T slab stays resident in SBUF for
  the residual adds. Only the final per-tile result is transposed (tensor
  engine) for the HBM store.
- After each layer an AllGather over the 8 cores rebuilds the full replica.
- Mean-pool: during the last layer, each [node, feat] output tile is reduced
  into a PSUM[feat, graph] accumulator via a matmul against a graph-selection
  one-hot built from a per-core batch-id input; scale by 1/count, AllReduce,
  then the tiny classifier MLP on every core.
"""
import os
import sys

sys.path.insert(0, "/opt/trn_rl_repo")

import numpy as np

N_NODES = 100000
N_EDGES = 1600000
F = 128
NUM_LAYERS = int(os.environ.get("GNN_LAYERS", "10"))
NUM_GRAPHS = 64
NUM_CLASSES = 2
N_CORES = 8
NPC = N_NODES // N_CORES          # 12500 nodes per core
NT = (NPC + 127) // 128           # 98 dst tiles per core
LAST_W = NPC - (NT - 1) * 128     # 84 nodes in the last tile

_CACHE = {}


def _preprocess(edge_index, batch):
    """Host-side: per-core gather indices / dst offsets in the SBUF layout."""
    src = np.ascontiguousarray(edge_index[0]).astype(np.int64)
    dst = np.ascontiguousarray(edge_index[1]).astype(np.int64)

    order = np.argsort(dst, kind="stable")
    s_src = src[order].astype(np.int32)
    s_dst = dst[order]

    # node-id boundaries of every (core, tile)
    tile_starts = (np.arange(N_CORES)[:, None] * NPC
                   + np.minimum(np.arange(NT + 1)[None, :] * 128, NPC))
    bounds = np.searchsorted(s_dst, tile_starts.ravel()).reshape(N_CORES, NT + 1)
    counts = np.diff(bounds, axis=1)                      # [N_CORES, NT]

    padded = np.maximum(counts.max(axis=0), 1)
    padded = ((padded + 127) // 128) * 128                # per-tile padded len
    nch = (padded // 128).astype(np.int64)                # 128-chunks per tile
    colbase = np.concatenate([[0], np.cumsum(nch)])
    C_total = int(colbase[-1])

    gidx = np.zeros((N_CORES, 128, C_total), dtype=np.int32)
    gdst = np.full((N_CORES, 128, C_total), -1.0, dtype=np.float32)
    for c in range(N_CORES):
        lo, hi = bounds[c, 0], bounds[c, NT]
        e_src = s_src[lo:hi]
        local = s_dst[lo:hi] - c * NPC
        t_e = local // 128
        grp_start = np.repeat(bounds[c, :-1] - lo, counts[c])
        r = np.arange(hi - lo) - grp_start                # rank within tile
        p = r % 128
        col = colbase[t_e] + r // 128
        gidx[c, p, col] = e_src
        gdst[c, p, col] = (local % 128).astype(np.float32)

    # per-core local batch ids, [128, NT], padding rows = -1
    blocal = np.full((N_CORES, 128, NT), -1.0, dtype=np.float32)
    b = np.asarray(batch).astype(np.int64)
    for c in range(N_CORES):
        ids = b[c * NPC:(c + 1) * NPC].astype(np.float32)
        ids = np.concatenate([ids, np.full(NT * 128 - NPC, -1.0, np.float32)])
        blocal[c] = ids.reshape(NT, 128).T

    cnt = np.bincount(b, minlength=NUM_GRAPHS).astype(np.float64)
    inv = (1.0 / np.maximum(cnt, 1.0)).astype(np.float32)
    return gidx, gdst, nch, colbase, blocal, inv


def _build(nch, colbase):
    from concourse import bacc, bass, mybir
    import concourse.tile as tile

    f32 = mybir.dt.float32
    C_total = int(colbase[-1])

    nc = bacc.Bacc("TRN2", target_bir_lowering=False, debug=False,
                   num_devices=N_CORES)

    # ---- I/O ----
    x_in = nc.dram_tensor("x", [N_NODES, F], f32, kind="ExternalInput")
    xT_in = nc.dram_tensor("xT_own", [F, NPC], f32, kind="ExternalInput")
    gidx_in = nc.dram_tensor("gidx", [128, C_total], mybir.dt.int32,
                             kind="ExternalInput")
    gdst_in = nc.dram_tensor("gdst", [128, C_total], f32, kind="ExternalInput")
    bl_in = nc.dram_tensor("blocal", [128, NT], f32, kind="ExternalInput")
    iota_in = nc.dram_tensor("iota", [128, 128], f32, kind="ExternalInput")
    ident_in = nc.dram_tensor("ident", [128, 128], f32, kind="ExternalInput")
    w1_in = nc.dram_tensor("w1cat", [F, NUM_LAYERS * F], f32, kind="ExternalInput")
    w2_in = nc.dram_tensor("w2cat", [F, NUM_LAYERS * F], f32, kind="ExternalInput")
    b1_in = nc.dram_tensor("b1t", [F, NUM_LAYERS], f32, kind="ExternalInput")
    b2_in = nc.dram_tensor("b2t", [F, NUM_LAYERS], f32, kind="ExternalInput")
    eps_in = nc.dram_tensor("epsb", [F, NUM_LAYERS], f32, kind="ExternalInput")
    wc1_in = nc.dram_tensor("wc1", [F, F], f32, kind="ExternalInput")
    bc1_in = nc.dram_tensor("bc1c", [F, 1], f32, kind="ExternalInput")
    wc2_in = nc.dram_tensor("wc2", [F, NUM_CLASSES], f32, kind="ExternalInput")
    bc2_in = nc.dram_tensor("bc2c", [NUM_CLASSES, 1], f32, kind="ExternalInput")
    inv_in = nc.dram_tensor("invc", [128, NUM_GRAPHS], f32, kind="ExternalInput")
    out_t = nc.dram_tensor("logits_t", [NUM_CLASSES, NUM_GRAPHS], f32,
                           kind="ExternalOutput")

    # ---- internal DRAM ----
    x_rep = nc.dram_tensor("x_rep", [N_NODES, F], f32, kind="Internal")
    newx = nc.dram_tensor("newx", [NPC, F], f32, kind="Internal")
    pr_in = nc.dram_tensor("pr_in", [128, NUM_GRAPHS], f32, kind="Internal")
    pr_out = nc.dram_tensor("pr_out", [128, NUM_GRAPHS], f32, kind="Internal")

    rg = [list(range(N_CORES))]

    with tile.TileContext(nc) as tc:
        from contextlib import ExitStack
        ctx = ExitStack()
        const = ctx.enter_context(tc.tile_pool(name="const", bufs=1))
        gpool = ctx.enter_context(tc.tile_pool(name="gather", bufs=3))
        opool = ctx.enter_context(tc.tile_pool(name="onehot", bufs=3))
        wpool = ctx.enter_context(tc.tile_pool(name="work", bufs=3))
        psum = ctx.enter_context(tc.tile_pool(name="psum", bufs=2, space="PSUM"))

        xT_res = const.tile([F, NPC], f32)
        gidx_t = const.tile([128, C_total], mybir.dt.int32)
        gdst_t = const.tile([128, C_total], f32)
        bl_t = const.tile([128, NT], f32)
        iota_t = const.tile([128, 128], f32)
        ident_t = const.tile([128, 128], f32)
        w1_t = const.tile([F, NUM_LAYERS * F], f32)
        w2_t = const.tile([F, NUM_LAYERS * F], f32)
        b1_t = const.tile([F, NUM_LAYERS], f32)
        b2_t = const.tile([F, NUM_LAYERS], f32)
        eps_t = const.tile([F, NUM_LAYERS], f32)
        wc1_t = const.tile([F, F], f32)
        bc1_t = const.tile([F, 1], f32)
        wc2_t = const.tile([F, NUM_CLASSES], f32)
        bc2_t = const.tile([NUM_CLASSES, 1], f32)
        inv_t = const.tile([128, NUM_GRAPHS], f32)

        for tle, src_t in [(xT_res, xT_in), (gdst_t, gdst_in), (bl_t, bl_in),
                           (iota_t, iota_in), (ident_t, ident_in),
                           (w1_t, w1_in), (w2_t, w2_in), (b1_t, b1_in),
                           (b2_t, b2_in), (eps_t, eps_in), (wc1_t, wc1_in),
                           (bc1_t, bc1_in), (wc2_t, wc2_in), (bc2_t, bc2_in),
                           (inv_t, inv_in)]:
            nc.sync.dma_start(tle[:], src_t[:])
        nc.sync.dma_start(gidx_t[:], gidx_in[:])

        pool_ps = psum.tile([F, NUM_GRAPHS], f32, tag="pool", bufs=1)

        for layer in range(NUM_LAYERS):
            src_dram = x_in if layer == 0 else x_rep
            last = layer == NUM_LAYERS - 1
            for t in range(NT):
                tw = 128 if t < NT - 1 else LAST_W
                n = int(nch[t])
                cb = int(colbase[t])
                ts = t * 128

                gbuf = gpool.tile([128, n, F], f32, tag="gbuf")
                for j in range(n):
                    # HW contract: one offset per partition, 128 rows/call
                    nc.gpsimd.indirect_dma_start(
                        out=gbuf[:, j, :],
                        out_offset=None,
                        in_=src_dram[:],
                        in_offset=bass.IndirectOffsetOnAxis(
                            ap=gidx_t[:, cb + j:cb + j + 1], axis=0),
                    )

                oh = opool.tile([128, n, 128], f32, tag="oh")
                nc.vector.tensor_tensor(
                    out=oh[:],
                    in0=gdst_t[:, cb:cb + n, None].to_broadcast([128, n, 128]),
                    in1=iota_t[:, None, :].to_broadcast([128, n, 128]),
                    op=mybir.AluOpType.is_equal,
                )

                aggr = psum.tile([F, 128], f32, tag="aggr", bufs=2)
                for j in range(n):
                    nc.tensor.matmul(aggr[:], gbuf[:, j, :], oh[:, j, :],
                                     start=(j == 0), stop=(j == n - 1))

                xT_sl = xT_res[:, ts:ts + tw]
                h = wpool.tile([F, 128], f32, tag="h")
                nc.vector.tensor_scalar(
                    out=h[:, :tw], in0=xT_sl, scalar1=eps_t[:, layer:layer + 1],
                    scalar2=None, op0=mybir.AluOpType.mult)
                nc.vector.tensor_tensor(
                    out=h[:, :tw], in0=h[:, :tw], in1=aggr[:, :tw],
                    op=mybir.AluOpType.add)

                p1 = psum.tile([F, 128], f32, tag="p1", bufs=1)
                nc.tensor.matmul(p1[:, :tw], w1_t[:, layer * F:(layer + 1) * F],
                                 h[:, :tw], start=True, stop=True)
                r1 = wpool.tile([F, 128], f32, tag="r1")
                nc.scalar.activation(r1[:, :tw], p1[:, :tw],
                                     mybir.ActivationFunctionType.Relu,
                                     bias=b1_t[:, layer:layer + 1])

                p2 = psum.tile([F, 128], f32, tag="p2", bufs=1)
                nc.tensor.matmul(p2[:, :tw], w2_t[:, layer * F:(layer + 1) * F],
                                 r1[:, :tw], start=True, stop=True)

                o = wpool.tile([F, 128], f32, tag="o")
                if layer > 0:
                    nc.vector.tensor_tensor(out=o[:, :tw], in0=p2[:, :tw],
                                            in1=h[:, :tw],
                                            op=mybir.AluOpType.add)
                    nc.scalar.activation(o[:, :tw], o[:, :tw],
                                         mybir.ActivationFunctionType.Relu,
                                         bias=b2_t[:, layer:layer + 1])
                else:
                    nc.scalar.activation(o[:, :tw], p2[:, :tw],
                                         mybir.ActivationFunctionType.Relu,
                                         bias=b2_t[:, layer:layer + 1])
                nc.vector.tensor_tensor(out=xT_sl, in0=o[:, :tw], in1=xT_sl,
                                        op=mybir.AluOpType.add)

                pt = psum.tile([128, F], f32, tag="pt", bufs=2)
                nc.tensor.transpose(out=pt[:tw, :], in_=xT_res[:, ts:ts + tw],
                                    identity=ident_t[:])
                st = wpool.tile([128, F], f32, tag="st")
                nc.vector.tensor_copy(st[:tw, :], pt[:tw, :])
                if not last:
                    nc.sync.dma_start(newx[ts:ts + tw, :], st[:tw, :])
                else:
                    # fold this tile into the pooling accumulator
                    sel = wpool.tile([128, NUM_GRAPHS], f32, tag="sel")
                    nc.vector.tensor_tensor(
                        out=sel[:],
                        in0=bl_t[:, t:t + 1].to_broadcast([128, NUM_GRAPHS]),
                        in1=iota_t[:, :NUM_GRAPHS],
                        op=mybir.AluOpType.is_equal,
                    )
                    nc.tensor.matmul(pool_ps[:], st[:], sel[:],
                                     start=(t == 0), stop=(t == NT - 1))

            if not last:
                nc.gpsimd.collective_compute(
                    "AllGather", mybir.AluOpType.bypass,
                    ins=[newx[:]], outs=[x_rep[:]], replica_groups=rg)

        # ---- mean pool + classifier ----
        pacc = wpool.tile([128, NUM_GRAPHS], f32, tag="pacc")
        nc.vector.tensor_tensor(out=pacc[:], in0=pool_ps[:], in1=inv_t[:],
                                op=mybir.AluOpType.mult)
        nc.sync.dma_start(pr_in[:], pacc[:])
        nc.gpsimd.collective_compute(
            "AllReduce", mybir.AluOpType.add,
            ins=[pr_in[:]], outs=[pr_out[:]], replica_groups=rg)
        pooled = wpool.tile([128, NUM_GRAPHS], f32, tag="pooled")
        nc.sync.dma_start(pooled[:], pr_out[:])

        pc1 = psum.tile([F, NUM_GRAPHS], f32, tag="aggr", bufs=2)
        nc.tensor.matmul(pc1[:], wc1_t[:], pooled[:], start=True, stop=True)
        rc1 = wpool.tile([F, NUM_GRAPHS], f32, tag="rc1")
        nc.scalar.activation(rc1[:], pc1[:], mybir.ActivationFunctionType.Relu,
                             bias=bc1_t[:])
        pc2 = psum.tile([NUM_CLASSES, NUM_GRAPHS], f32, tag="p1", bufs=1)
        nc.tensor.matmul(pc2[:], wc2_t[:], rc1[:], start=True, stop=True)
        lg = wpool.tile([NUM_CLASSES, NUM_GRAPHS], f32, tag="lg")
        nc.vector.tensor_scalar(out=lg[:], in0=pc2[:], scalar1=bc2_t[:],
                                scalar2=None, op0=mybir.AluOpType.add)
        nc.sync.dma_start(out_t[:], lg[:])
        ctx.close()

    nc.compile()
    return nc


def _get_module(nch, colbase):
    key = tuple(nch.tolist())
    if key not in _CACHE:
        _CACHE.clear()
        _CACHE[key] = _build(nch, colbase)
    return _CACHE[key]


def kernel(x, edge_index, batch, eps, W1, b1, W2, b2, Wc1, bc1, Wc2, bc2,
           _trace=False):
    from concourse.bass_utils import run_bass_kernel_spmd

    x = np.ascontiguousarray(np.asarray(x), dtype=np.float32)
    eps = np.asarray(eps, dtype=np.float32)
    W1 = np.asarray(W1, dtype=np.float32)
    b1 = np.asarray(b1, dtype=np.float32)
    W2 = np.asarray(W2, dtype=np.float32)
    b2 = np.asarray(b2, dtype=np.float32)

    gidx, gdst, nch, colbase, blocal, inv = _preprocess(
        np.asarray(edge_index), np.asarray(batch))
    nc = _get_module(nch, colbase)

    L = NUM_LAYERS
    common = {
        "x": x,
        "iota": np.ascontiguousarray(
            np.broadcast_to(np.arange(128, dtype=np.float32), (128, 128))),
        "ident": np.eye(128, dtype=np.float32),
        "w1cat": np.ascontiguousarray(np.concatenate(list(W1[:L]), axis=1)),
        "w2cat": np.ascontiguousarray(np.concatenate(list(W2[:L]), axis=1)),
        "b1t": np.ascontiguousarray(b1[:L].T),
        "b2t": np.ascontiguousarray(b2[:L].T),
        "epsb": np.ascontiguousarray(
            np.broadcast_to(1.0 + eps[:L], (F, L))),
        "wc1": np.ascontiguousarray(np.asarray(Wc1, np.float32)),
        "bc1c": np.ascontiguousarray(np.asarray(bc1, np.float32)[:, None]),
        "wc2": np.ascontiguousarray(np.asarray(Wc2, np.float32)),
        "bc2c": np.ascontiguousarray(np.asarray(bc2, np.float32)[:, None]),
        "invc": np.ascontiguousarray(np.broadcast_to(inv, (128, NUM_GRAPHS))),
    }
    in_maps = []
    for c in range(N_CORES):
        m = dict(common)
        m["xT_own"] = np.ascontiguousarray(x[c * NPC:(c + 1) * NPC].T)
        m["gidx"] = gidx[c]
        m["gdst"] = gdst[c]
        m["blocal"] = blocal[c]
        in_maps.append(m)

    res = run_bass_kernel_spmd(nc, in_maps, core_ids=list(range(N_CORES)),
                               trace=_trace)
    out = np.ascontiguousarray(res.results[0]["logits_t"].T)
    if _trace:
        kernel._last_result = res
    return out



# revision 8
# speedup vs baseline: 1.1026x; 1.1026x over previous
"""GIN message-passing GNN on 8 Trainium2 NeuronCores (Bass/Tile).

Strategy (self-contained; shapes hardcoded for the 100k-node / 1.6M-edge /
128-dim / 10-layer / 64-graph problem):

- Nodes are partitioned into 8 contiguous ranges of 12500; each core owns the
  edges whose *destination* falls in its range.
- Each core keeps a full bf16 replica of the node features in its HBM. Per
  layer it gathers x[src] for its ~200k edges with ONE batched indirect DMA
  per 128-dst tile (offset AP [128, n] -> 128*n rows per call), with edges
  pre-sorted by dst tile host-side and padded to a multiple of 128 (max over
  cores so the program is SPMD).
- The segment-sum (scatter-add) becomes a matmul: for each 128-edge chunk,
  PSUM[feat, dst] += contract_edges(gathered[edge, feat], onehot[edge, dst]);
  the one-hot is built in bf16 on the vector engine (2x mode) by a broadcast
  is_equal against a resident iota row. Padding lanes have dstoff=-1.
- The GIN MLP runs in bf16 in the transposed [feat, node] orientation; the
  fp32 per-core x^T slab stays resident in SBUF for the residual adds.
- The replica is rebuilt after each layer by a *chunked* AllGather (5 row
  groups) into a double-buffered replica laid out group-major
  [g][core][row][feat], so collectives overlap the next groups' compute.
- Mean-pool: during the last layer each [node, feat] output tile is reduced
  into PSUM[feat, graph] via a matmul against a graph one-hot; scale by
  1/count, AllReduce, then the tiny classifier MLP on every core.
"""
import os
import sys

sys.path.insert(0, "/opt/trn_rl_repo")

import numpy as np

N_NODES = 100000
N_EDGES = 1600000
F = 128
NUM_LAYERS = int(os.environ.get("GNN_LAYERS", "10"))
NUM_GRAPHS = 64
NUM_CLASSES = 2
N_CORES = 8
NPC = N_NODES // N_CORES          # 12500 nodes per core
NT = (NPC + 127) // 128           # 98 dst tiles per core
LAST_W = NPC - (NT - 1) * 128     # 84 nodes in the last tile

# AllGather chunking: groups of dst tiles whose newx rows are exchanged as
# soon as they are computed. 5 groups of ~20 tiles.
_N_GROUPS = int(os.environ.get("GNN_AG_GROUPS", "5"))
if _N_GROUPS == 1:
    GROUP_TILES = [98]
else:
    GROUP_TILES = [20, 20, 20, 20, 18]
BATCH_GATHER = os.environ.get("GNN_BATCH", "0") == "1"
SHARED_REP = os.environ.get("GNN_SHARED", "0") == "1"
assert sum(GROUP_TILES) == NT
GROUP_ROWS = [min(t * 128, NPC) - min(s * 128, NPC)
              for t, s in zip(np.cumsum(GROUP_TILES),
                              np.cumsum([0] + GROUP_TILES[:-1]))]
GROUP_OFF = np.concatenate([[0], np.cumsum(GROUP_ROWS)]).astype(np.int64)

_CACHE = {}


def _node_to_rep_row(node):
    """Map global node id -> row in the group-major replica layout."""
    c = node // NPC
    r = node % NPC
    g = np.searchsorted(GROUP_OFF, r, side="right") - 1
    lg = GROUP_OFF[g + 1] - GROUP_OFF[g]
    return N_CORES * GROUP_OFF[g] + c * lg + (r - GROUP_OFF[g])


def _preprocess(edge_index, batch):
    """Host-side: per-core gather indices / dst offsets in the SBUF layout."""
    src = np.ascontiguousarray(edge_index[0]).astype(np.int64)
    dst = np.ascontiguousarray(edge_index[1]).astype(np.int64)

    order = np.argsort(dst, kind="stable")
    s_src = src[order]
    s_dst = dst[order]
    s_src_rep = _node_to_rep_row(s_src).astype(np.int32)

    # node-id boundaries of every (core, tile)
    tile_starts = (np.arange(N_CORES)[:, None] * NPC
                   + np.minimum(np.arange(NT + 1)[None, :] * 128, NPC))
    bounds = np.searchsorted(s_dst, tile_starts.ravel()).reshape(N_CORES, NT + 1)
    counts = np.diff(bounds, axis=1)                      # [N_CORES, NT]

    padded = np.maximum(counts.max(axis=0), 1)
    padded = ((padded + 127) // 128) * 128                # per-tile padded len
    nch = (padded // 128).astype(np.int64)                # 128-chunks per tile
    colbase = np.concatenate([[0], np.cumsum(nch)])
    C_total = int(colbase[-1])

    gidx = np.zeros((N_CORES, 128, C_total), dtype=np.int32)
    gdst = np.full((N_CORES, 128, C_total), -1.0, dtype=np.float32)
    for c in range(N_CORES):
        lo, hi = bounds[c, 0], bounds[c, NT]
        e_src = s_src_rep[lo:hi]
        local = s_dst[lo:hi] - c * NPC
        t_e = local // 128
        grp_start = np.repeat(bounds[c, :-1] - lo, counts[c])
        r = np.arange(hi - lo) - grp_start                # rank within tile
        p = r % 128
        col = colbase[t_e] + r // 128
        gidx[c, p, col] = e_src
        gdst[c, p, col] = (local % 128).astype(np.float32)

    # per-core local batch ids, [128, NT], padding rows = -1
    blocal = np.full((N_CORES, 128, NT), -1.0, dtype=np.float32)
    b = np.asarray(batch).astype(np.int64)
    for c in range(N_CORES):
        ids = b[c * NPC:(c + 1) * NPC].astype(np.float32)
        ids = np.concatenate([ids, np.full(NT * 128 - NPC, -1.0, np.float32)])
        blocal[c] = ids.reshape(NT, 128).T

    cnt = np.bincount(b, minlength=NUM_GRAPHS).astype(np.float64)
    inv = (1.0 / np.maximum(cnt, 1.0)).astype(np.float32)
    return gidx, gdst, nch, colbase, blocal, inv


def _build(nch, colbase):
    from concourse import bacc, bass, mybir
    import concourse.tile as tile

    f32 = mybir.dt.float32
    bf16 = mybir.dt.bfloat16
    C_total = int(colbase[-1])

    nc = bacc.Bacc("TRN2", target_bir_lowering=False, debug=False,
                   num_devices=N_CORES)

    # ---- I/O ----
    xrep0_in = nc.dram_tensor("xrep0", [N_NODES, F], bf16, kind="ExternalInput")
    xT_in = nc.dram_tensor("xT_own", [F, NPC], f32, kind="ExternalInput")
    gidx_in = nc.dram_tensor("gidx", [128, C_total], mybir.dt.int32,
                             kind="ExternalInput")
    gdst_in = nc.dram_tensor("gdst", [128, C_total], bf16, kind="ExternalInput")
    bl_in = nc.dram_tensor("blocal", [128, NT], f32, kind="ExternalInput")
    iota_in = nc.dram_tensor("iota", [128, 128], bf16, kind="ExternalInput")
    iotaf_in = nc.dram_tensor("iotaf", [128, NUM_GRAPHS], f32,
                              kind="ExternalInput")
    ident_in = nc.dram_tensor("ident", [128, 128], f32, kind="ExternalInput")
    w1_in = nc.dram_tensor("w1cat", [F, NUM_LAYERS * F], bf16,
                           kind="ExternalInput")
    w2_in = nc.dram_tensor("w2cat", [F, NUM_LAYERS * F], bf16,
                           kind="ExternalInput")
    b1_in = nc.dram_tensor("b1t", [F, NUM_LAYERS], f32, kind="ExternalInput")
    b2_in = nc.dram_tensor("b2t", [F, NUM_LAYERS], f32, kind="ExternalInput")
    eps_in = nc.dram_tensor("epsb", [F, NUM_LAYERS], f32, kind="ExternalInput")
    wc1_in = nc.dram_tensor("wc1", [F, F], f32, kind="ExternalInput")
    bc1_in = nc.dram_tensor("bc1c", [F, 1], f32, kind="ExternalInput")
    wc2_in = nc.dram_tensor("wc2", [F, NUM_CLASSES], f32, kind="ExternalInput")
    bc2_in = nc.dram_tensor("bc2c", [NUM_CLASSES, 1], f32, kind="ExternalInput")
    inv_in = nc.dram_tensor("invc", [128, NUM_GRAPHS], f32, kind="ExternalInput")
    out_t = nc.dram_tensor("logits_t", [NUM_CLASSES, NUM_GRAPHS], f32,
                           kind="ExternalOutput")

    # ---- internal DRAM ----
    # double-buffered group-major replica [g][core][row][feat]
    _space = "Shared" if SHARED_REP else "Local"
    x_rep = [nc.dram_tensor(f"x_rep{i}", [N_NODES, F], bf16, kind="Internal",
                            addr_space=_space)
             for i in range(2)]
    newx = nc.dram_tensor("newx", [NPC, F], bf16, kind="Internal")
    pr_in = nc.dram_tensor("pr_in", [128, NUM_GRAPHS], f32, kind="Internal")
    pr_out = nc.dram_tensor("pr_out", [128, NUM_GRAPHS], f32, kind="Internal",
                            addr_space="Shared")

    rg = [list(range(N_CORES))]
    n_groups = len(GROUP_TILES)
    group_first = np.cumsum([0] + GROUP_TILES[:-1])
    group_last = np.cumsum(GROUP_TILES) - 1

    with tile.TileContext(nc) as tc:
        from contextlib import ExitStack
        ctx = ExitStack()
        const = ctx.enter_context(tc.tile_pool(name="const", bufs=1))
        gpool = ctx.enter_context(tc.tile_pool(name="gather", bufs=3))
        opool = ctx.enter_context(tc.tile_pool(name="onehot", bufs=3))
        wpool = ctx.enter_context(tc.tile_pool(name="work", bufs=3))
        psum = ctx.enter_context(tc.tile_pool(name="psum", bufs=2, space="PSUM"))

        xT_res = const.tile([F, NPC], f32)
        gidx_t = const.tile([128, C_total], mybir.dt.int32)
        gdst_t = const.tile([128, C_total], bf16)
        bl_t = const.tile([128, NT], f32)
        iota_t = const.tile([128, 128], bf16)
        iotaf_t = const.tile([128, NUM_GRAPHS], f32)
        ident_t = const.tile([128, 128], f32)
        w1_t = const.tile([F, NUM_LAYERS * F], bf16)
        w2_t = const.tile([F, NUM_LAYERS * F], bf16)
        b1_t = const.tile([F, NUM_LAYERS], f32)
        b2_t = const.tile([F, NUM_LAYERS], f32)
        eps_t = const.tile([F, NUM_LAYERS], f32)
        wc1_t = const.tile([F, F], f32)
        bc1_t = const.tile([F, 1], f32)
        wc2_t = const.tile([F, NUM_CLASSES], f32)
        bc2_t = const.tile([NUM_CLASSES, 1], f32)
        inv_t = const.tile([128, NUM_GRAPHS], f32)

        for tle, src_t in [(xT_res, xT_in), (gdst_t, gdst_in), (bl_t, bl_in),
                           (iota_t, iota_in), (iotaf_t, iotaf_in),
                           (ident_t, ident_in),
                           (w1_t, w1_in), (w2_t, w2_in), (b1_t, b1_in),
                           (b2_t, b2_in), (eps_t, eps_in), (wc1_t, wc1_in),
                           (bc1_t, bc1_in), (wc2_t, wc2_in), (bc2_t, bc2_in),
                           (inv_t, inv_in)]:
            nc.sync.dma_start(tle[:], src_t[:])
        nc.sync.dma_start(gidx_t[:], gidx_in[:])

        pool_ps = psum.tile([F, NUM_GRAPHS], f32, tag="pool", bufs=1)

        for layer in range(NUM_LAYERS):
            src_dram = xrep0_in if layer == 0 else x_rep[(layer - 1) % 2]
            dst_rep = x_rep[layer % 2]
            last = layer == NUM_LAYERS - 1
            for t in range(NT):
                tw = 128 if t < NT - 1 else LAST_W
                n = int(nch[t])
                cb = int(colbase[t])
                ts = t * 128

                # one batched indirect DMA: 128*n rows
                gbuf = gpool.tile([128, n, F], bf16, tag="gbuf")
                if BATCH_GATHER:
                    nc.gpsimd.indirect_dma_start(
                        out=gbuf[:, :, :],
                        out_offset=None,
                        in_=src_dram[:],
                        in_offset=bass.IndirectOffsetOnAxis(
                            ap=gidx_t[:, cb:cb + n], axis=0),
                    )
                else:
                    for j in range(n):
                        nc.gpsimd.indirect_dma_start(
                            out=gbuf[:, j, :],
                            out_offset=None,
                            in_=src_dram[:],
                            in_offset=bass.IndirectOffsetOnAxis(
                                ap=gidx_t[:, cb + j:cb + j + 1], axis=0),
                        )

                oh = opool.tile([128, n, 128], bf16, tag="oh")
                nc.vector.tensor_tensor(
                    out=oh[:],
                    in0=gdst_t[:, cb:cb + n, None].to_broadcast([128, n, 128]),
                    in1=iota_t[:, None, :].to_broadcast([128, n, 128]),
                    op=mybir.AluOpType.is_equal,
                )

                aggr = psum.tile([F, 128], f32, tag="aggr", bufs=2)
                for j in range(n):
                    nc.tensor.matmul(aggr[:], gbuf[:, j, :], oh[:, j, :],
                                     start=(j == 0), stop=(j == n - 1))

                xT_sl = xT_res[:, ts:ts + tw]
                # h in fp32 (for mlp-residual) and bf16 (for matmul input)
                hf = wpool.tile([F, 128], f32, tag="hf")
                nc.vector.tensor_scalar(
                    out=hf[:, :tw], in0=xT_sl, scalar1=eps_t[:, layer:layer + 1],
                    scalar2=None, op0=mybir.AluOpType.mult)
                nc.vector.tensor_tensor(
                    out=hf[:, :tw], in0=hf[:, :tw], in1=aggr[:, :tw],
                    op=mybir.AluOpType.add)
                h = wpool.tile([F, 128], bf16, tag="h")
                nc.scalar.activation(h[:, :tw], hf[:, :tw],
                                     mybir.ActivationFunctionType.Copy)

                p1 = psum.tile([F, 128], f32, tag="p1", bufs=1)
                nc.tensor.matmul(p1[:, :tw], w1_t[:, layer * F:(layer + 1) * F],
                                 h[:, :tw], start=True, stop=True)
                r1 = wpool.tile([F, 128], bf16, tag="r1")
                nc.scalar.activation(r1[:, :tw], p1[:, :tw],
                                     mybir.ActivationFunctionType.Relu,
                                     bias=b1_t[:, layer:layer + 1])

                p2 = psum.tile([F, 128], f32, tag="p2", bufs=1)
                nc.tensor.matmul(p2[:, :tw], w2_t[:, layer * F:(layer + 1) * F],
                                 r1[:, :tw], start=True, stop=True)

                o = wpool.tile([F, 128], f32, tag="o")
                if layer > 0:
                    nc.vector.tensor_tensor(out=o[:, :tw], in0=p2[:, :tw],
                                            in1=hf[:, :tw],
                                            op=mybir.AluOpType.add)
                    nc.scalar.activation(o[:, :tw], o[:, :tw],
                                         mybir.ActivationFunctionType.Relu,
                                         bias=b2_t[:, layer:layer + 1])
                else:
                    nc.scalar.activation(o[:, :tw], p2[:, :tw],
                                         mybir.ActivationFunctionType.Relu,
                                         bias=b2_t[:, layer:layer + 1])
                nc.vector.tensor_tensor(out=xT_sl, in0=o[:, :tw], in1=xT_sl,
                                        op=mybir.AluOpType.add)

                if not last:
                    # transpose the updated fp32 slab, cast to bf16 on store
                    pt = psum.tile([128, F], f32, tag="pt", bufs=2)
                    nc.tensor.transpose(out=pt[:tw, :], in_=xT_res[:, ts:ts + tw],
                                        identity=ident_t[:])
                    st = wpool.tile([128, F], bf16, tag="st")
                    nc.scalar.activation(st[:tw, :], pt[:tw, :],
                                         mybir.ActivationFunctionType.Copy)
                    nc.sync.dma_start(newx[ts:ts + tw, :], st[:tw, :])
                else:
                    # fold this tile into the pooling accumulator
                    pt = psum.tile([128, F], f32, tag="pt", bufs=2)
                    nc.tensor.transpose(out=pt[:tw, :], in_=xT_res[:, ts:ts + tw],
                                        identity=ident_t[:])
                    st = wpool.tile([128, F], f32, tag="st")
                    nc.vector.tensor_copy(st[:tw, :], pt[:tw, :])
                    sel = wpool.tile([128, NUM_GRAPHS], f32, tag="sel")
                    nc.vector.tensor_tensor(
                        out=sel[:],
                        in0=bl_t[:, t:t + 1].to_broadcast([128, NUM_GRAPHS]),
                        in1=iotaf_t[:],
                        op=mybir.AluOpType.is_equal,
                    )
                    nc.tensor.matmul(pool_ps[:], st[:], sel[:],
                                     start=(t == 0), stop=(t == NT - 1))

                if (not last) and t in group_last:
                    g = int(np.searchsorted(group_last, t))
                    s_row = int(GROUP_OFF[g])
                    l_row = int(GROUP_ROWS[g])
                    nc.gpsimd.collective_compute(
                        "AllGather", mybir.AluOpType.bypass,
                        ins=[newx[s_row:s_row + l_row, :]],
                        outs=[dst_rep[N_CORES * s_row:
                                      N_CORES * s_row + N_CORES * l_row, :]],
                        replica_groups=rg)

        # ---- mean pool + classifier ----
        pacc = wpool.tile([128, NUM_GRAPHS], f32, tag="pacc")
        nc.vector.tensor_tensor(out=pacc[:], in0=pool_ps[:], in1=inv_t[:],
                                op=mybir.AluOpType.mult)
        nc.sync.dma_start(pr_in[:], pacc[:])
        nc.gpsimd.collective_compute(
            "AllReduce", mybir.AluOpType.add,
            ins=[pr_in[:]], outs=[pr_out[:]], replica_groups=rg)
        pooled = wpool.tile([128, NUM_GRAPHS], f32, tag="pooled")
        nc.sync.dma_start(pooled[:], pr_out[:])

        pc1 = psum.tile([F, NUM_GRAPHS], f32, tag="aggr", bufs=2)
        nc.tensor.matmul(pc1[:], wc1_t[:], pooled[:], start=True, stop=True)
        rc1 = wpool.tile([F, NUM_GRAPHS], f32, tag="rc1")
        nc.scalar.activation(rc1[:], pc1[:], mybir.ActivationFunctionType.Relu,
                             bias=bc1_t[:])
        pc2 = psum.tile([NUM_CLASSES, NUM_GRAPHS], f32, tag="p1", bufs=1)
        nc.tensor.matmul(pc2[:], wc2_t[:], rc1[:], start=True, stop=True)
        lg = wpool.tile([NUM_CLASSES, NUM_GRAPHS], f32, tag="lg")
        nc.vector.tensor_scalar(out=lg[:], in0=pc2[:], scalar1=bc2_t[:],
                                scalar2=None, op0=mybir.AluOpType.add)
        nc.sync.dma_start(out_t[:], lg[:])
        ctx.close()

    nc.compile()
    return nc


def _get_module(nch, colbase):
    key = tuple(nch.tolist())
    if key not in _CACHE:
        _CACHE.clear()
        _CACHE[key] = _build(nch, colbase)
    return _CACHE[key]


def kernel(x, edge_index, batch, eps, W1, b1, W2, b2, Wc1, bc1, Wc2, bc2,
           _trace=False):
    import ml_dtypes
    from concourse.bass_utils import run_bass_kernel_spmd

    x = np.ascontiguousarray(np.asarray(x), dtype=np.float32)
    eps = np.asarray(eps, dtype=np.float32)
    W1 = np.asarray(W1, dtype=np.float32)
    b1 = np.asarray(b1, dtype=np.float32)
    W2 = np.asarray(W2, dtype=np.float32)
    b2 = np.asarray(b2, dtype=np.float32)

    gidx, gdst, nch, colbase, blocal, inv = _preprocess(
        np.asarray(edge_index), np.asarray(batch))
    nc = _get_module(nch, colbase)

    # group-major bf16 replica of the initial x
    xrep0 = np.empty((N_NODES, F), dtype=ml_dtypes.bfloat16)
    x_by_core = x.reshape(N_CORES, NPC, F)
    for g in range(len(GROUP_TILES)):
        s, e = int(GROUP_OFF[g]), int(GROUP_OFF[g + 1])
        xrep0[N_CORES * s:N_CORES * e] = (
            x_by_core[:, s:e, :].reshape(-1, F).astype(ml_dtypes.bfloat16))

    L = NUM_LAYERS
    common = {
        "xrep0": xrep0,
        "iota": np.ascontiguousarray(np.broadcast_to(
            np.arange(128, dtype=np.float32), (128, 128))).astype(
                ml_dtypes.bfloat16),
        "iotaf": np.ascontiguousarray(np.broadcast_to(
            np.arange(NUM_GRAPHS, dtype=np.float32), (128, NUM_GRAPHS))),
        "ident": np.eye(128, dtype=np.float32),
        "w1cat": np.ascontiguousarray(
            np.concatenate(list(W1[:L]), axis=1)).astype(ml_dtypes.bfloat16),
        "w2cat": np.ascontiguousarray(
            np.concatenate(list(W2[:L]), axis=1)).astype(ml_dtypes.bfloat16),
        "b1t": np.ascontiguousarray(b1[:L].T),
        "b2t": np.ascontiguousarray(b2[:L].T),
        "epsb": np.ascontiguousarray(
            np.broadcast_to(1.0 + eps[:L], (F, L))),
        "wc1": np.ascontiguousarray(np.asarray(Wc1, np.float32)),
        "bc1c": np.ascontiguousarray(np.asarray(bc1, np.float32)[:, None]),
        "wc2": np.ascontiguousarray(np.asarray(Wc2, np.float32)),
        "bc2c": np.ascontiguousarray(np.asarray(bc2, np.float32)[:, None]),
        "invc": np.ascontiguousarray(np.broadcast_to(inv, (128, NUM_GRAPHS))),
    }
    in_maps = []
    for c in range(N_CORES):
        m = dict(common)
        m["xT_own"] = np.ascontiguousarray(x[c * NPC:(c + 1) * NPC].T)
        m["gidx"] = gidx[c]
        m["gdst"] = gdst[c].astype(ml_dtypes.bfloat16)
        m["blocal"] = blocal[c]
        in_maps.append(m)

    res = run_bass_kernel_spmd(nc, in_maps, core_ids=list(range(N_CORES)),
                               trace=_trace)
    out = np.ascontiguousarray(res.results[0]["logits_t"].T)
    if _trace:
        kernel._last_result = res
    return out


# revision 20
# speedup vs baseline: 1.1040x; 1.0013x over previous
"""GIN message-passing GNN on 8 Trainium2 NeuronCores (Bass/Tile).

Strategy (self-contained; shapes hardcoded for the 100k-node / 1.6M-edge /
128-dim / 10-layer / 64-graph problem):

- Nodes are partitioned into 8 contiguous ranges of 12500; each core owns the
  edges whose *destination* falls in its range.
- Each core keeps a full bf16 replica of the node features in its HBM. Per
  layer it gathers x[src] for its ~200k edges with ONE batched indirect DMA
  per 128-dst tile (offset AP [128, n] -> 128*n rows per call), with edges
  pre-sorted by dst tile host-side and padded to a multiple of 128 (max over
  cores so the program is SPMD).
- The segment-sum (scatter-add) becomes a matmul: for each 128-edge chunk,
  PSUM[feat, dst] += contract_edges(gathered[edge, feat], onehot[edge, dst]);
  the one-hot is built in bf16 on the vector engine (2x mode) by a broadcast
  is_equal against a resident iota row. Padding lanes have dstoff=-1.
- The GIN MLP runs in bf16 in the transposed [feat, node] orientation; the
  fp32 per-core x^T slab stays resident in SBUF for the residual adds.
- The replica is rebuilt after each layer by a *chunked* AllGather (5 row
  groups) into a double-buffered replica laid out group-major
  [g][core][row][feat], so collectives overlap the next groups' compute.
- Mean-pool: during the last layer each [node, feat] output tile is reduced
  into PSUM[feat, graph] via a matmul against a graph one-hot; scale by
  1/count, AllReduce, then the tiny classifier MLP on every core.
"""
import os
import sys

sys.path.insert(0, "/opt/trn_rl_repo")

import numpy as np

N_NODES = 100000
N_EDGES = 1600000
F = 128
NUM_LAYERS = int(os.environ.get("GNN_LAYERS", "10"))
NUM_GRAPHS = 64
NUM_CLASSES = 2
N_CORES = 8
NPC = N_NODES // N_CORES          # 12500 nodes per core
NT = (NPC + 127) // 128           # 98 dst tiles per core
LAST_W = NPC - (NT - 1) * 128     # 84 nodes in the last tile

# AllGather chunking: groups of dst tiles whose newx rows are exchanged as
# soon as they are computed. 5 groups of ~20 tiles.
_N_GROUPS = int(os.environ.get("GNN_AG_GROUPS", "5"))
if _N_GROUPS == 1:
    GROUP_TILES = [98]
else:
    GROUP_TILES = [20, 20, 20, 20, 18]
BATCH_GATHER = os.environ.get("GNN_BATCH", "0") == "1"
SHARED_REP = os.environ.get("GNN_SHARED", "0") == "1"
# dma_gather (InstDMAGatherAnt) path: crashes the device (NRT status 101)
# in this environment — keep off.
GATHER_ANT = os.environ.get("GNN_ANT", "0") == "1"
N_BUCKETS = 4 if GATHER_ANT else 1
BUCKET_ROWS = 25000 if GATHER_ANT else N_NODES        # int16 idx < 32768
assert sum(GROUP_TILES) == NT
GROUP_ROWS = [min(t * 128, NPC) - min(s * 128, NPC)
              for t, s in zip(np.cumsum(GROUP_TILES),
                              np.cumsum([0] + GROUP_TILES[:-1]))]
GROUP_OFF = np.concatenate([[0], np.cumsum(GROUP_ROWS)]).astype(np.int64)

_CACHE = {}


def _node_to_rep_row(node):
    """Map global node id -> row in the group-major replica layout."""
    c = node // NPC
    r = node % NPC
    g = np.searchsorted(GROUP_OFF, r, side="right") - 1
    lg = GROUP_OFF[g + 1] - GROUP_OFF[g]
    return N_CORES * GROUP_OFF[g] + c * lg + (r - GROUP_OFF[g])


def _preprocess(edge_index, batch):
    """Host-side: per-core gather indices / dst offsets in the SBUF layout.

    Edges are sorted by (core, dst-tile, src-bucket, dst). Chunk columns of
    each tile are partitioned by src bucket so one dma_gather per
    (tile, bucket) fills a contiguous column range; int16 indices are
    bucket-local rows. Padding slots keep gdst=-1 (one-hot zeroes them) and
    gather row 0 of the bucket.
    """
    src = np.ascontiguousarray(edge_index[0]).astype(np.int64)
    dst = np.ascontiguousarray(edge_index[1]).astype(np.int64)

    src_rep = _node_to_rep_row(src)
    bucket = src_rep // BUCKET_ROWS
    gtile = (dst // NPC) * NT + (dst % NPC) // 128        # global tile id
    order = np.lexsort((dst, bucket, gtile))
    s_rep = src_rep[order]
    s_dst = dst[order]
    s_b = bucket[order]
    s_gt = gtile[order]

    # counts per (core, tile, bucket)
    cell = s_gt * N_BUCKETS + s_b
    counts = np.bincount(cell, minlength=N_CORES * NT * N_BUCKETS).reshape(
        N_CORES, NT, N_BUCKETS)

    nchb = np.ceil(counts.max(axis=0) / 128).astype(np.int64)  # [NT, NB]
    nchb[0, 0] = max(nchb[0, 0], 1)
    nch = nchb.sum(axis=1)                                # chunks per tile
    colbase = np.concatenate([[0], np.cumsum(nch)])
    C_total = int(colbase[-1])
    # column offset of each (t, b) block
    cbt = np.concatenate(
        [np.zeros((NT, 1), np.int64), np.cumsum(nchb, axis=1)], axis=1)
    cb_tb = colbase[:-1, None] + cbt[:, :-1]              # [NT, NB]

    gidx = np.zeros((N_CORES, 128, C_total), dtype=np.int32)
    idx16 = np.zeros((N_CORES, 128, 8 * C_total), dtype=np.int16)
    gdst = np.full((N_CORES, 128, C_total), -1.0, dtype=np.float32)
    cell_starts = np.concatenate([[0], np.cumsum(counts.ravel())])
    for c in range(N_CORES):
        lo = cell_starts[c * NT * N_BUCKETS]
        hi = cell_starts[(c + 1) * NT * N_BUCKETS]
        e_rep = s_rep[lo:hi]
        local = s_dst[lo:hi] - c * NPC
        t_e = (local // 128).astype(np.int64)
        b_e = s_b[lo:hi]
        # rank within the (t, b) cell
        cell_c = (t_e * N_BUCKETS + b_e)
        grp_start = cell_starts[c * NT * N_BUCKETS:(c + 1) * NT * N_BUCKETS]
        r = np.arange(hi - lo) - (grp_start[cell_c] - lo)
        p = r % 128
        col = cb_tb[t_e, b_e] + r // 128
        gidx[c, p, col] = e_rep.astype(np.int32)
        gdst[c, p, col] = (local % 128).astype(np.float32)
        # int16 bucket-local indices, packed [r%16, 8*cb + r//16] per block
        blk_r = (col - cb_tb[t_e, b_e]) * 128 + p         # == r
        idx16[c, blk_r % 16, 8 * cb_tb[t_e, b_e] + blk_r // 16] = \
            (e_rep - b_e * BUCKET_ROWS).astype(np.int16)

    # per-core local batch ids, [128, NT], padding rows = -1
    blocal = np.full((N_CORES, 128, NT), -1.0, dtype=np.float32)
    b = np.asarray(batch).astype(np.int64)
    for c in range(N_CORES):
        ids = b[c * NPC:(c + 1) * NPC].astype(np.float32)
        ids = np.concatenate([ids, np.full(NT * 128 - NPC, -1.0, np.float32)])
        blocal[c] = ids.reshape(NT, 128).T

    cnt = np.bincount(b, minlength=NUM_GRAPHS).astype(np.float64)
    inv = (1.0 / np.maximum(cnt, 1.0)).astype(np.float32)
    return gidx, idx16, gdst, nch, nchb, colbase, blocal, inv


def _build(nch, nchb, colbase):
    from concourse import bacc, bass, mybir, library_config
    import concourse.tile as tile

    f32 = mybir.dt.float32
    bf16 = mybir.dt.bfloat16
    C_total = int(colbase[-1])

    nc = bacc.Bacc("TRN2", target_bir_lowering=False, debug=False,
                   num_devices=N_CORES)

    # ---- I/O ----
    xrep0_in = nc.dram_tensor("xrep0", [N_NODES, F], bf16, kind="ExternalInput")
    xT_in = nc.dram_tensor("xT_own", [F, NPC], f32, kind="ExternalInput")
    gidx_in = nc.dram_tensor("gidx", [128, C_total], mybir.dt.int32,
                             kind="ExternalInput")
    idx16_in = nc.dram_tensor("idx16", [128, 8 * C_total], mybir.dt.int16,
                              kind="ExternalInput")
    gdst_in = nc.dram_tensor("gdst", [128, C_total], bf16, kind="ExternalInput")
    bl_in = nc.dram_tensor("blocal", [128, NT], f32, kind="ExternalInput")
    iota_in = nc.dram_tensor("iota", [128, 128], bf16, kind="ExternalInput")
    iotaf_in = nc.dram_tensor("iotaf", [128, NUM_GRAPHS], f32,
                              kind="ExternalInput")
    ident_in = nc.dram_tensor("ident", [128, 128], f32, kind="ExternalInput")
    w1_in = nc.dram_tensor("w1cat", [F, NUM_LAYERS * F], bf16,
                           kind="ExternalInput")
    w2_in = nc.dram_tensor("w2cat", [F, NUM_LAYERS * F], bf16,
                           kind="ExternalInput")
    b1_in = nc.dram_tensor("b1t", [F, NUM_LAYERS], f32, kind="ExternalInput")
    b2_in = nc.dram_tensor("b2t", [F, NUM_LAYERS], f32, kind="ExternalInput")
    eps_in = nc.dram_tensor("epsb", [F, NUM_LAYERS], f32, kind="ExternalInput")
    wc1_in = nc.dram_tensor("wc1", [F, F], f32, kind="ExternalInput")
    bc1_in = nc.dram_tensor("bc1c", [F, 1], f32, kind="ExternalInput")
    wc2_in = nc.dram_tensor("wc2", [F, NUM_CLASSES], f32, kind="ExternalInput")
    bc2_in = nc.dram_tensor("bc2c", [NUM_CLASSES, 1], f32, kind="ExternalInput")
    inv_in = nc.dram_tensor("invc", [128, NUM_GRAPHS], f32, kind="ExternalInput")
    out_t = nc.dram_tensor("logits_t", [NUM_CLASSES, NUM_GRAPHS], f32,
                           kind="ExternalOutput")

    # ---- internal DRAM ----
    # double-buffered group-major replica [g][core][row][feat]
    _space = "Shared" if SHARED_REP else "Local"
    x_rep = [nc.dram_tensor(f"x_rep{i}", [N_NODES, F], bf16, kind="Internal",
                            addr_space=_space)
             for i in range(2)]
    newx = nc.dram_tensor("newx", [NPC, F], bf16, kind="Internal")
    pr_in = nc.dram_tensor("pr_in", [128, NUM_GRAPHS], f32, kind="Internal")
    pr_out = nc.dram_tensor("pr_out", [128, NUM_GRAPHS], f32, kind="Internal",
                            addr_space="Shared")

    rg = [list(range(N_CORES))]
    n_groups = len(GROUP_TILES)
    group_first = np.cumsum([0] + GROUP_TILES[:-1])
    group_last = np.cumsum(GROUP_TILES) - 1

    with tile.TileContext(nc) as tc:
        from contextlib import ExitStack
        ctx = ExitStack()
        const = ctx.enter_context(tc.tile_pool(name="const", bufs=1))
        gpool = ctx.enter_context(tc.tile_pool(name="gather", bufs=3))
        opool = ctx.enter_context(tc.tile_pool(name="onehot", bufs=3))
        wpool = ctx.enter_context(tc.tile_pool(name="work", bufs=3))
        psum = ctx.enter_context(tc.tile_pool(name="psum", bufs=2, space="PSUM"))

        if GATHER_ANT:
            nc.gpsimd.load_library(library_config.mlp)

        xT_res = const.tile([F, NPC], f32)
        gidx_t = const.tile([128, C_total], mybir.dt.int32)
        idx16_t = const.tile([128, 8 * C_total], mybir.dt.int16)
        gdst_t = const.tile([128, C_total], bf16)
        bl_t = const.tile([128, NT], f32)
        iota_t = const.tile([128, 128], bf16)
        iotaf_t = const.tile([128, NUM_GRAPHS], f32)
        ident_t = const.tile([128, 128], f32)
        w1_t = const.tile([F, NUM_LAYERS * F], bf16)
        w2_t = const.tile([F, NUM_LAYERS * F], bf16)
        b1_t = const.tile([F, NUM_LAYERS], f32)
        b2_t = const.tile([F, NUM_LAYERS], f32)
        eps_t = const.tile([F, NUM_LAYERS], f32)
        wc1_t = const.tile([F, F], f32)
        bc1_t = const.tile([F, 1], f32)
        wc2_t = const.tile([F, NUM_CLASSES], f32)
        bc2_t = const.tile([NUM_CLASSES, 1], f32)
        inv_t = const.tile([128, NUM_GRAPHS], f32)

        for tle, src_t in [(xT_res, xT_in), (gdst_t, gdst_in), (bl_t, bl_in),
                           (iota_t, iota_in), (iotaf_t, iotaf_in),
                           (ident_t, ident_in),
                           (w1_t, w1_in), (w2_t, w2_in), (b1_t, b1_in),
                           (b2_t, b2_in), (eps_t, eps_in), (wc1_t, wc1_in),
                           (bc1_t, bc1_in), (wc2_t, wc2_in), (bc2_t, bc2_in),
                           (inv_t, inv_in)]:
            nc.sync.dma_start(tle[:], src_t[:])
        nc.sync.dma_start(gidx_t[:], gidx_in[:])
        nc.sync.dma_start(idx16_t[:], idx16_in[:])

        pool_ps = psum.tile([F, NUM_GRAPHS], f32, tag="pool", bufs=1)

        for layer in range(NUM_LAYERS):
            src_dram = xrep0_in if layer == 0 else x_rep[(layer - 1) % 2]
            dst_rep = x_rep[layer % 2]
            last = layer == NUM_LAYERS - 1
            for t in range(NT):
                tw = 128 if t < NT - 1 else LAST_W
                n = int(nch[t])
                cb = int(colbase[t])
                ts = t * 128

                gbuf = gpool.tile([128, n, F], bf16, tag="gbuf")
                if GATHER_ANT:
                    # one dma_gather per (tile, src-bucket)
                    o = 0
                    for b in range(N_BUCKETS):
                        n_tb = int(nchb[t, b])
                        if n_tb == 0:
                            continue
                        cbb = cb + o
                        nc.gpsimd.dma_gather(
                            gbuf[:, o:o + n_tb, :],
                            src_dram[b * BUCKET_ROWS:(b + 1) * BUCKET_ROWS, :],
                            idx16_t[:, 8 * cbb:8 * (cbb + n_tb)],
                            128 * n_tb,
                            128 * n_tb,
                            F,
                        )
                        o += n_tb
                elif BATCH_GATHER:
                    nc.gpsimd.indirect_dma_start(
                        out=gbuf[:, :, :],
                        out_offset=None,
                        in_=src_dram[:],
                        in_offset=bass.IndirectOffsetOnAxis(
                            ap=gidx_t[:, cb:cb + n], axis=0),
                    )
                else:
                    for j in range(n):
                        nc.gpsimd.indirect_dma_start(
                            out=gbuf[:, j, :],
                            out_offset=None,
                            in_=src_dram[:],
                            in_offset=bass.IndirectOffsetOnAxis(
                                ap=gidx_t[:, cb + j:cb + j + 1], axis=0),
                        )

                oh = opool.tile([128, n, 128], bf16, tag="oh")
                nc.vector.tensor_tensor(
                    out=oh[:],
                    in0=gdst_t[:, cb:cb + n, None].to_broadcast([128, n, 128]),
                    in1=iota_t[:, None, :].to_broadcast([128, n, 128]),
                    op=mybir.AluOpType.is_equal,
                )

                aggr = psum.tile([F, 128], f32, tag="aggr", bufs=2)
                for j in range(n):
                    nc.tensor.matmul(aggr[:], gbuf[:, j, :], oh[:, j, :],
                                     start=(j == 0), stop=(j == n - 1))

                xT_sl = xT_res[:, ts:ts + tw]
                # h in fp32 (for mlp-residual) and bf16 (for matmul input)
                hf = wpool.tile([F, 128], f32, tag="hf")
                nc.vector.tensor_scalar(
                    out=hf[:, :tw], in0=xT_sl, scalar1=eps_t[:, layer:layer + 1],
                    scalar2=None, op0=mybir.AluOpType.mult)
                nc.vector.tensor_tensor(
                    out=hf[:, :tw], in0=hf[:, :tw], in1=aggr[:, :tw],
                    op=mybir.AluOpType.add)
                h = wpool.tile([F, 128], bf16, tag="h")
                nc.scalar.activation(h[:, :tw], hf[:, :tw],
                                     mybir.ActivationFunctionType.Copy)

                p1 = psum.tile([F, 128], f32, tag="p1", bufs=1)
                nc.tensor.matmul(p1[:, :tw], w1_t[:, layer * F:(layer + 1) * F],
                                 h[:, :tw], start=True, stop=True)
                r1 = wpool.tile([F, 128], bf16, tag="r1")
                nc.scalar.activation(r1[:, :tw], p1[:, :tw],
                                     mybir.ActivationFunctionType.Relu,
                                     bias=b1_t[:, layer:layer + 1])

                p2 = psum.tile([F, 128], f32, tag="p2", bufs=1)
                nc.tensor.matmul(p2[:, :tw], w2_t[:, layer * F:(layer + 1) * F],
                                 r1[:, :tw], start=True, stop=True)

                o = wpool.tile([F, 128], f32, tag="o")
                if layer > 0:
                    nc.vector.tensor_tensor(out=o[:, :tw], in0=p2[:, :tw],
                                            in1=hf[:, :tw],
                                            op=mybir.AluOpType.add)
                    nc.scalar.activation(o[:, :tw], o[:, :tw],
                                         mybir.ActivationFunctionType.Relu,
                                         bias=b2_t[:, layer:layer + 1])
                else:
                    nc.scalar.activation(o[:, :tw], p2[:, :tw],
                                         mybir.ActivationFunctionType.Relu,
                                         bias=b2_t[:, layer:layer + 1])
                nc.vector.tensor_tensor(out=xT_sl, in0=o[:, :tw], in1=xT_sl,
                                        op=mybir.AluOpType.add)

                if not last:
                    # transpose the updated fp32 slab, cast to bf16 on store
                    pt = psum.tile([128, F], f32, tag="pt", bufs=2)
                    nc.tensor.transpose(out=pt[:tw, :], in_=xT_res[:, ts:ts + tw],
                                        identity=ident_t[:])
                    st = wpool.tile([128, F], bf16, tag="st")
                    nc.scalar.activation(st[:tw, :], pt[:tw, :],
                                         mybir.ActivationFunctionType.Copy)
                    nc.sync.dma_start(newx[ts:ts + tw, :], st[:tw, :])
                else:
                    # fold this tile into the pooling accumulator
                    pt = psum.tile([128, F], f32, tag="pt", bufs=2)
                    nc.tensor.transpose(out=pt[:tw, :], in_=xT_res[:, ts:ts + tw],
                                        identity=ident_t[:])
                    st = wpool.tile([128, F], f32, tag="st")
                    nc.vector.tensor_copy(st[:tw, :], pt[:tw, :])
                    sel = wpool.tile([128, NUM_GRAPHS], f32, tag="sel")
                    nc.vector.tensor_tensor(
                        out=sel[:],
                        in0=bl_t[:, t:t + 1].to_broadcast([128, NUM_GRAPHS]),
                        in1=iotaf_t[:],
                        op=mybir.AluOpType.is_equal,
                    )
                    nc.tensor.matmul(pool_ps[:], st[:], sel[:],
                                     start=(t == 0), stop=(t == NT - 1))

                if (not last) and t in group_last:
                    g = int(np.searchsorted(group_last, t))
                    s_row = int(GROUP_OFF[g])
                    l_row = int(GROUP_ROWS[g])
                    nc.gpsimd.collective_compute(
                        "AllGather", mybir.AluOpType.bypass,
                        ins=[newx[s_row:s_row + l_row, :]],
                        outs=[dst_rep[N_CORES * s_row:
                                      N_CORES * s_row + N_CORES * l_row, :]],
                        replica_groups=rg)

        # ---- mean pool + classifier ----
        pacc = wpool.tile([128, NUM_GRAPHS], f32, tag="pacc")
        nc.vector.tensor_tensor(out=pacc[:], in0=pool_ps[:], in1=inv_t[:],
                                op=mybir.AluOpType.mult)
        nc.sync.dma_start(pr_in[:], pacc[:])
        nc.gpsimd.collective_compute(
            "AllReduce", mybir.AluOpType.add,
            ins=[pr_in[:]], outs=[pr_out[:]], replica_groups=rg)
        pooled = wpool.tile([128, NUM_GRAPHS], f32, tag="pooled")
        nc.sync.dma_start(pooled[:], pr_out[:])

        pc1 = psum.tile([F, NUM_GRAPHS], f32, tag="aggr", bufs=2)
        nc.tensor.matmul(pc1[:], wc1_t[:], pooled[:], start=True, stop=True)
        rc1 = wpool.tile([F, NUM_GRAPHS], f32, tag="rc1")
        nc.scalar.activation(rc1[:], pc1[:], mybir.ActivationFunctionType.Relu,
                             bias=bc1_t[:])
        pc2 = psum.tile([NUM_CLASSES, NUM_GRAPHS], f32, tag="p1", bufs=1)
        nc.tensor.matmul(pc2[:], wc2_t[:], rc1[:], start=True, stop=True)
        lg = wpool.tile([NUM_CLASSES, NUM_GRAPHS], f32, tag="lg")
        nc.vector.tensor_scalar(out=lg[:], in0=pc2[:], scalar1=bc2_t[:],
                                scalar2=None, op0=mybir.AluOpType.add)
        nc.sync.dma_start(out_t[:], lg[:])
        ctx.close()

    nc.compile()
    return nc


def _get_module(nch, nchb, colbase):
    key = tuple(nch.tolist()) + tuple(nchb.ravel().tolist())
    if key not in _CACHE:
        _CACHE.clear()
        _CACHE[key] = _build(nch, nchb, colbase)
    return _CACHE[key]


def kernel(x, edge_index, batch, eps, W1, b1, W2, b2, Wc1, bc1, Wc2, bc2,
           _trace=False):
    import ml_dtypes
    from concourse.bass_utils import run_bass_kernel_spmd

    x = np.ascontiguousarray(np.asarray(x), dtype=np.float32)
    eps = np.asarray(eps, dtype=np.float32)
    W1 = np.asarray(W1, dtype=np.float32)
    b1 = np.asarray(b1, dtype=np.float32)
    W2 = np.asarray(W2, dtype=np.float32)
    b2 = np.asarray(b2, dtype=np.float32)

    gidx, idx16, gdst, nch, nchb, colbase, blocal, inv = _preprocess(
        np.asarray(edge_index), np.asarray(batch))
    nc = _get_module(nch, nchb, colbase)

    # group-major bf16 replica of the initial x
    xrep0 = np.empty((N_NODES, F), dtype=ml_dtypes.bfloat16)
    x_by_core = x.reshape(N_CORES, NPC, F)
    for g in range(len(GROUP_TILES)):
        s, e = int(GROUP_OFF[g]), int(GROUP_OFF[g + 1])
        xrep0[N_CORES * s:N_CORES * e] = (
            x_by_core[:, s:e, :].reshape(-1, F).astype(ml_dtypes.bfloat16))

    L = NUM_LAYERS
    common = {
        "xrep0": xrep0,
        "iota": np.ascontiguousarray(np.broadcast_to(
            np.arange(128, dtype=np.float32), (128, 128))).astype(
                ml_dtypes.bfloat16),
        "iotaf": np.ascontiguousarray(np.broadcast_to(
            np.arange(NUM_GRAPHS, dtype=np.float32), (128, NUM_GRAPHS))),
        "ident": np.eye(128, dtype=np.float32),
        "w1cat": np.ascontiguousarray(
            np.concatenate(list(W1[:L]), axis=1)).astype(ml_dtypes.bfloat16),
        "w2cat": np.ascontiguousarray(
            np.concatenate(list(W2[:L]), axis=1)).astype(ml_dtypes.bfloat16),
        "b1t": np.ascontiguousarray(b1[:L].T),
        "b2t": np.ascontiguousarray(b2[:L].T),
        "epsb": np.ascontiguousarray(
            np.broadcast_to(1.0 + eps[:L], (F, L))),
        "wc1": np.ascontiguousarray(np.asarray(Wc1, np.float32)),
        "bc1c": np.ascontiguousarray(np.asarray(bc1, np.float32)[:, None]),
        "wc2": np.ascontiguousarray(np.asarray(Wc2, np.float32)),
        "bc2c": np.ascontiguousarray(np.asarray(bc2, np.float32)[:, None]),
        "invc": np.ascontiguousarray(np.broadcast_to(inv, (128, NUM_GRAPHS))),
    }
    in_maps = []
    for c in range(N_CORES):
        m = dict(common)
        m["xT_own"] = np.ascontiguousarray(x[c * NPC:(c + 1) * NPC].T)
        m["gidx"] = gidx[c]
        m["idx16"] = idx16[c]
        m["gdst"] = gdst[c].astype(ml_dtypes.bfloat16)
        m["blocal"] = blocal[c]
        in_maps.append(m)

    res = run_bass_kernel_spmd(nc, in_maps, core_ids=list(range(N_CORES)),
                               trace=_trace)
    out = np.ascontiguousarray(res.results[0]["logits_t"].T)
    if _trace:
        kernel._last_result = res
    return out


# revision 23
# speedup vs baseline: 1.1966x; 1.0840x over previous
"""GIN message-passing GNN on 8 Trainium2 NeuronCores (Bass/Tile).

Strategy (self-contained; shapes hardcoded for the 100k-node / 1.6M-edge /
128-dim / 10-layer / 64-graph problem):

- Nodes are partitioned into 8 contiguous ranges of 12500; each core owns the
  edges whose *destination* falls in its range.
- Each core keeps a full bf16 replica of the node features in its HBM. Per
  layer it gathers x[src] for its ~200k edges with ONE batched indirect DMA
  per 128-dst tile (offset AP [128, n] -> 128*n rows per call), with edges
  pre-sorted by dst tile host-side and padded to a multiple of 128 (max over
  cores so the program is SPMD).
- The segment-sum (scatter-add) becomes a matmul: for each 128-edge chunk,
  PSUM[feat, dst] += contract_edges(gathered[edge, feat], onehot[edge, dst]);
  the one-hot is built in bf16 on the vector engine (2x mode) by a broadcast
  is_equal against a resident iota row. Padding lanes have dstoff=-1.
- The GIN MLP runs in bf16 in the transposed [feat, node] orientation; the
  fp32 per-core x^T slab stays resident in SBUF for the residual adds.
- The replica is rebuilt after each layer by a *chunked* AllGather (5 row
  groups) into a double-buffered replica laid out group-major
  [g][core][row][feat], so collectives overlap the next groups' compute.
- Mean-pool: during the last layer each [node, feat] output tile is reduced
  into PSUM[feat, graph] via a matmul against a graph one-hot; scale by
  1/count, AllReduce, then the tiny classifier MLP on every core.
"""
import os
import sys

sys.path.insert(0, "/opt/trn_rl_repo")

import numpy as np

N_NODES = 100000
N_EDGES = 1600000
F = 128
NUM_LAYERS = int(os.environ.get("GNN_LAYERS", "10"))
NUM_GRAPHS = 64
NUM_CLASSES = 2
N_CORES = 8
NPC = N_NODES // N_CORES          # 12500 nodes per core
NT = (NPC + 127) // 128           # 98 dst tiles per core
LAST_W = NPC - (NT - 1) * 128     # 84 nodes in the last tile

# AllGather chunking: groups of dst tiles whose newx rows are exchanged as
# soon as they are computed. 5 groups of ~20 tiles.
_N_GROUPS = int(os.environ.get("GNN_AG_GROUPS", "5"))
if _N_GROUPS == 1:
    GROUP_TILES = [98]
else:
    GROUP_TILES = [20, 20, 20, 20, 18]
BATCH_GATHER = os.environ.get("GNN_BATCH", "0") == "1"
SHARED_REP = os.environ.get("GNN_SHARED", "0") == "1"
# dma_gather (InstDMAGatherAnt) path: crashes the device (NRT status 101)
# in this environment — keep off.
GATHER_ANT = os.environ.get("GNN_ANT", "0") == "1"
N_BUCKETS = 4 if GATHER_ANT else 1
BUCKET_ROWS = 25000 if GATHER_ANT else N_NODES        # int16 idx < 32768
assert sum(GROUP_TILES) == NT
GROUP_ROWS = [min(t * 128, NPC) - min(s * 128, NPC)
              for t, s in zip(np.cumsum(GROUP_TILES),
                              np.cumsum([0] + GROUP_TILES[:-1]))]
GROUP_OFF = np.concatenate([[0], np.cumsum(GROUP_ROWS)]).astype(np.int64)

_CACHE = {}


def _node_to_rep_row(node):
    """Map global node id -> row in the group-major replica layout."""
    c = node // NPC
    r = node % NPC
    g = np.searchsorted(GROUP_OFF, r, side="right") - 1
    lg = GROUP_OFF[g + 1] - GROUP_OFF[g]
    return N_CORES * GROUP_OFF[g] + c * lg + (r - GROUP_OFF[g])


def _preprocess(edge_index, batch):
    """Host-side: per-core gather indices / dst offsets in the SBUF layout.

    Edges are sorted by (core, dst-tile, src-bucket, dst). Chunk columns of
    each tile are partitioned by src bucket so one dma_gather per
    (tile, bucket) fills a contiguous column range; int16 indices are
    bucket-local rows. Padding slots keep gdst=-1 (one-hot zeroes them) and
    gather row 0 of the bucket.
    """
    src = np.ascontiguousarray(edge_index[0]).astype(np.int64)
    dst = np.ascontiguousarray(edge_index[1]).astype(np.int64)

    src_rep = _node_to_rep_row(src)
    bucket = src_rep // BUCKET_ROWS
    gtile = (dst // NPC) * NT + (dst % NPC) // 128        # global tile id
    order = np.lexsort((dst, bucket, gtile))
    s_rep = src_rep[order]
    s_dst = dst[order]
    s_b = bucket[order]
    s_gt = gtile[order]

    # counts per (core, tile, bucket)
    cell = s_gt * N_BUCKETS + s_b
    counts = np.bincount(cell, minlength=N_CORES * NT * N_BUCKETS).reshape(
        N_CORES, NT, N_BUCKETS)

    nchb = np.ceil(counts.max(axis=0) / 128).astype(np.int64)  # [NT, NB]
    nchb[0, 0] = max(nchb[0, 0], 1)
    nch = nchb.sum(axis=1)                                # chunks per tile
    colbase = np.concatenate([[0], np.cumsum(nch)])
    C_total = int(colbase[-1])
    # column offset of each (t, b) block
    cbt = np.concatenate(
        [np.zeros((NT, 1), np.int64), np.cumsum(nchb, axis=1)], axis=1)
    cb_tb = colbase[:-1, None] + cbt[:, :-1]              # [NT, NB]

    gidx = np.zeros((N_CORES, 128, C_total), dtype=np.int32)
    idx16 = np.zeros((N_CORES, 128, 8 * C_total), dtype=np.int16)
    gdst = np.full((N_CORES, 128, C_total), -1.0, dtype=np.float32)
    cell_starts = np.concatenate([[0], np.cumsum(counts.ravel())])
    for c in range(N_CORES):
        lo = cell_starts[c * NT * N_BUCKETS]
        hi = cell_starts[(c + 1) * NT * N_BUCKETS]
        e_rep = s_rep[lo:hi]
        local = s_dst[lo:hi] - c * NPC
        t_e = (local // 128).astype(np.int64)
        b_e = s_b[lo:hi]
        # rank within the (t, b) cell
        cell_c = (t_e * N_BUCKETS + b_e)
        grp_start = cell_starts[c * NT * N_BUCKETS:(c + 1) * NT * N_BUCKETS]
        r = np.arange(hi - lo) - (grp_start[cell_c] - lo)
        p = r % 128
        col = cb_tb[t_e, b_e] + r // 128
        gidx[c, p, col] = e_rep.astype(np.int32)
        gdst[c, p, col] = (local % 128).astype(np.float32)
        # int16 bucket-local indices, packed [r%16, 8*cb + r//16] per block
        blk_r = (col - cb_tb[t_e, b_e]) * 128 + p         # == r
        idx16[c, blk_r % 16, 8 * cb_tb[t_e, b_e] + blk_r // 16] = \
            (e_rep - b_e * BUCKET_ROWS).astype(np.int16)
    # HW contract: the 16-partition index stripe is replicated to all 8
    # GpSimd cores' partition stripes (sim reads only [:16])
    idx16 = np.tile(idx16[:, :16, :], (1, 8, 1))

    # per-core local batch ids, [128, NT], padding rows = -1
    blocal = np.full((N_CORES, 128, NT), -1.0, dtype=np.float32)
    b = np.asarray(batch).astype(np.int64)
    for c in range(N_CORES):
        ids = b[c * NPC:(c + 1) * NPC].astype(np.float32)
        ids = np.concatenate([ids, np.full(NT * 128 - NPC, -1.0, np.float32)])
        blocal[c] = ids.reshape(NT, 128).T

    cnt = np.bincount(b, minlength=NUM_GRAPHS).astype(np.float64)
    inv = (1.0 / np.maximum(cnt, 1.0)).astype(np.float32)
    return gidx, idx16, gdst, nch, nchb, colbase, blocal, inv


def _build(nch, nchb, colbase):
    from concourse import bacc, bass, mybir, library_config
    import concourse.tile as tile

    f32 = mybir.dt.float32
    bf16 = mybir.dt.bfloat16
    C_total = int(colbase[-1])

    nc = bacc.Bacc("TRN2", target_bir_lowering=False, debug=False,
                   num_devices=N_CORES)

    # ---- I/O ----
    xrep0_in = nc.dram_tensor("xrep0", [N_NODES, F], bf16, kind="ExternalInput")
    xT_in = nc.dram_tensor("xT_own", [F, NPC], f32, kind="ExternalInput")
    gidx_in = nc.dram_tensor("gidx", [128, C_total], mybir.dt.int32,
                             kind="ExternalInput")
    idx16_in = nc.dram_tensor("idx16", [128, 8 * C_total], mybir.dt.int16,
                              kind="ExternalInput")
    gdst_in = nc.dram_tensor("gdst", [128, C_total], bf16, kind="ExternalInput")
    bl_in = nc.dram_tensor("blocal", [128, NT], f32, kind="ExternalInput")
    iota_in = nc.dram_tensor("iota", [128, 128], bf16, kind="ExternalInput")
    iotaf_in = nc.dram_tensor("iotaf", [128, NUM_GRAPHS], f32,
                              kind="ExternalInput")
    ident_in = nc.dram_tensor("ident", [128, 128], f32, kind="ExternalInput")
    w1_in = nc.dram_tensor("w1cat", [F, NUM_LAYERS * F], bf16,
                           kind="ExternalInput")
    w2_in = nc.dram_tensor("w2cat", [F, NUM_LAYERS * F], bf16,
                           kind="ExternalInput")
    b1_in = nc.dram_tensor("b1t", [F, NUM_LAYERS], f32, kind="ExternalInput")
    b2_in = nc.dram_tensor("b2t", [F, NUM_LAYERS], f32, kind="ExternalInput")
    eps_in = nc.dram_tensor("epsb", [F, NUM_LAYERS], f32, kind="ExternalInput")
    wc1_in = nc.dram_tensor("wc1", [F, F], f32, kind="ExternalInput")
    bc1_in = nc.dram_tensor("bc1c", [F, 1], f32, kind="ExternalInput")
    wc2_in = nc.dram_tensor("wc2", [F, NUM_CLASSES], f32, kind="ExternalInput")
    bc2_in = nc.dram_tensor("bc2c", [NUM_CLASSES, 1], f32, kind="ExternalInput")
    inv_in = nc.dram_tensor("invc", [128, NUM_GRAPHS], f32, kind="ExternalInput")
    out_t = nc.dram_tensor("logits_t", [NUM_CLASSES, NUM_GRAPHS], f32,
                           kind="ExternalOutput")

    # ---- internal DRAM ----
    # double-buffered group-major replica [g][core][row][feat]
    _space = "Shared" if SHARED_REP else "Local"
    x_rep = [nc.dram_tensor(f"x_rep{i}", [N_NODES, F], bf16, kind="Internal",
                            addr_space=_space)
             for i in range(2)]
    newx = nc.dram_tensor("newx", [NPC, F], bf16, kind="Internal")
    pr_in = nc.dram_tensor("pr_in", [128, NUM_GRAPHS], f32, kind="Internal")
    pr_out = nc.dram_tensor("pr_out", [128, NUM_GRAPHS], f32, kind="Internal",
                            addr_space="Shared")

    rg = [list(range(N_CORES))]
    n_groups = len(GROUP_TILES)
    group_first = np.cumsum([0] + GROUP_TILES[:-1])
    group_last = np.cumsum(GROUP_TILES) - 1

    with tile.TileContext(nc) as tc:
        from contextlib import ExitStack
        ctx = ExitStack()
        const = ctx.enter_context(tc.tile_pool(name="const", bufs=1))
        gpool = ctx.enter_context(tc.tile_pool(name="gather", bufs=3))
        opool = ctx.enter_context(tc.tile_pool(name="onehot", bufs=3))
        wpool = ctx.enter_context(tc.tile_pool(name="work", bufs=3))
        psum = ctx.enter_context(tc.tile_pool(name="psum", bufs=2, space="PSUM"))

        lib_inst = None
        if GATHER_ANT:
            # Tile may otherwise schedule gathers before the Q7 library load
            lib_inst = nc.gpsimd.load_library(library_config.mlp)

        xT_res = const.tile([F, NPC], f32)
        gidx_t = const.tile([128, C_total], mybir.dt.int32)
        idx16_t = const.tile([128, 8 * C_total], mybir.dt.int16)
        gdst_t = const.tile([128, C_total], bf16)
        bl_t = const.tile([128, NT], f32)
        iota_t = const.tile([128, 128], bf16)
        iotaf_t = const.tile([128, NUM_GRAPHS], f32)
        ident_t = const.tile([128, 128], f32)
        w1_t = const.tile([F, NUM_LAYERS * F], bf16)
        w2_t = const.tile([F, NUM_LAYERS * F], bf16)
        b1_t = const.tile([F, NUM_LAYERS], f32)
        b2_t = const.tile([F, NUM_LAYERS], f32)
        eps_t = const.tile([F, NUM_LAYERS], f32)
        wc1_t = const.tile([F, F], f32)
        bc1_t = const.tile([F, 1], f32)
        wc2_t = const.tile([F, NUM_CLASSES], f32)
        bc2_t = const.tile([NUM_CLASSES, 1], f32)
        inv_t = const.tile([128, NUM_GRAPHS], f32)

        for tle, src_t in [(xT_res, xT_in), (gdst_t, gdst_in), (bl_t, bl_in),
                           (iota_t, iota_in), (iotaf_t, iotaf_in),
                           (ident_t, ident_in),
                           (w1_t, w1_in), (w2_t, w2_in), (b1_t, b1_in),
                           (b2_t, b2_in), (eps_t, eps_in), (wc1_t, wc1_in),
                           (bc1_t, bc1_in), (wc2_t, wc2_in), (bc2_t, bc2_in),
                           (inv_t, inv_in)]:
            nc.sync.dma_start(tle[:], src_t[:])
        nc.sync.dma_start(gidx_t[:], gidx_in[:])
        nc.sync.dma_start(idx16_t[:], idx16_in[:])

        pool_ps = psum.tile([F, NUM_GRAPHS], f32, tag="pool", bufs=1)

        for layer in range(NUM_LAYERS):
            src_dram = xrep0_in if layer == 0 else x_rep[(layer - 1) % 2]
            dst_rep = x_rep[layer % 2]
            last = layer == NUM_LAYERS - 1
            for t in range(NT):
                tw = 128 if t < NT - 1 else LAST_W
                n = int(nch[t])
                cb = int(colbase[t])
                ts = t * 128

                gbuf = gpool.tile([128, n, F], bf16, tag="gbuf")
                if GATHER_ANT:
                    # one dma_gather per (tile, src-bucket)
                    o = 0
                    for b in range(N_BUCKETS):
                        n_tb = int(nchb[t, b])
                        if n_tb == 0:
                            continue
                        cbb = cb + o
                        g = nc.gpsimd.dma_gather(
                            gbuf[:, o:o + n_tb, :],
                            src_dram[b * BUCKET_ROWS:(b + 1) * BUCKET_ROWS, :],
                            idx16_t[:, 8 * cbb:8 * (cbb + n_tb)],
                            128 * n_tb,
                            128 * n_tb,
                            F,
                            single_packet=False,
                        )
                        bass._add_dep_helper(
                            g.ins, lib_inst.ins, sync=True,
                            reason="ucode library before extended-inst gather")
                        o += n_tb
                elif BATCH_GATHER:
                    nc.gpsimd.indirect_dma_start(
                        out=gbuf[:, :, :],
                        out_offset=None,
                        in_=src_dram[:],
                        in_offset=bass.IndirectOffsetOnAxis(
                            ap=gidx_t[:, cb:cb + n], axis=0),
                    )
                else:
                    for j in range(n):
                        nc.gpsimd.indirect_dma_start(
                            out=gbuf[:, j, :],
                            out_offset=None,
                            in_=src_dram[:],
                            in_offset=bass.IndirectOffsetOnAxis(
                                ap=gidx_t[:, cb + j:cb + j + 1], axis=0),
                        )

                oh = opool.tile([128, n, 128], bf16, tag="oh")
                nc.vector.tensor_tensor(
                    out=oh[:],
                    in0=gdst_t[:, cb:cb + n, None].to_broadcast([128, n, 128]),
                    in1=iota_t[:, None, :].to_broadcast([128, n, 128]),
                    op=mybir.AluOpType.is_equal,
                )

                aggr = psum.tile([F, 128], f32, tag="aggr", bufs=2)
                for j in range(n):
                    nc.tensor.matmul(aggr[:], gbuf[:, j, :], oh[:, j, :],
                                     start=(j == 0), stop=(j == n - 1))

                xT_sl = xT_res[:, ts:ts + tw]
                # h in fp32 (for mlp-residual) and bf16 (for matmul input)
                hf = wpool.tile([F, 128], f32, tag="hf")
                nc.vector.tensor_scalar(
                    out=hf[:, :tw], in0=xT_sl, scalar1=eps_t[:, layer:layer + 1],
                    scalar2=None, op0=mybir.AluOpType.mult)
                nc.vector.tensor_tensor(
                    out=hf[:, :tw], in0=hf[:, :tw], in1=aggr[:, :tw],
                    op=mybir.AluOpType.add)
                h = wpool.tile([F, 128], bf16, tag="h")
                nc.scalar.activation(h[:, :tw], hf[:, :tw],
                                     mybir.ActivationFunctionType.Copy)

                p1 = psum.tile([F, 128], f32, tag="p1", bufs=1)
                nc.tensor.matmul(p1[:, :tw], w1_t[:, layer * F:(layer + 1) * F],
                                 h[:, :tw], start=True, stop=True)
                r1 = wpool.tile([F, 128], bf16, tag="r1")
                nc.scalar.activation(r1[:, :tw], p1[:, :tw],
                                     mybir.ActivationFunctionType.Relu,
                                     bias=b1_t[:, layer:layer + 1])

                p2 = psum.tile([F, 128], f32, tag="p2", bufs=1)
                nc.tensor.matmul(p2[:, :tw], w2_t[:, layer * F:(layer + 1) * F],
                                 r1[:, :tw], start=True, stop=True)

                o = wpool.tile([F, 128], f32, tag="o")
                if layer > 0:
                    nc.vector.tensor_tensor(out=o[:, :tw], in0=p2[:, :tw],
                                            in1=hf[:, :tw],
                                            op=mybir.AluOpType.add)
                    nc.scalar.activation(o[:, :tw], o[:, :tw],
                                         mybir.ActivationFunctionType.Relu,
                                         bias=b2_t[:, layer:layer + 1])
                else:
                    nc.scalar.activation(o[:, :tw], p2[:, :tw],
                                         mybir.ActivationFunctionType.Relu,
                                         bias=b2_t[:, layer:layer + 1])
                nc.vector.tensor_tensor(out=xT_sl, in0=o[:, :tw], in1=xT_sl,
                                        op=mybir.AluOpType.add)

                if not last:
                    # transpose the updated fp32 slab, cast to bf16 on store
                    pt = psum.tile([128, F], f32, tag="pt", bufs=2)
                    nc.tensor.transpose(out=pt[:tw, :], in_=xT_res[:, ts:ts + tw],
                                        identity=ident_t[:])
                    st = wpool.tile([128, F], bf16, tag="st")
                    nc.scalar.activation(st[:tw, :], pt[:tw, :],
                                         mybir.ActivationFunctionType.Copy)
                    nc.sync.dma_start(newx[ts:ts + tw, :], st[:tw, :])
                else:
                    # fold this tile into the pooling accumulator
                    pt = psum.tile([128, F], f32, tag="pt", bufs=2)
                    nc.tensor.transpose(out=pt[:tw, :], in_=xT_res[:, ts:ts + tw],
                                        identity=ident_t[:])
                    st = wpool.tile([128, F], f32, tag="st")
                    nc.vector.tensor_copy(st[:tw, :], pt[:tw, :])
                    sel = wpool.tile([128, NUM_GRAPHS], f32, tag="sel")
                    nc.vector.tensor_tensor(
                        out=sel[:],
                        in0=bl_t[:, t:t + 1].to_broadcast([128, NUM_GRAPHS]),
                        in1=iotaf_t[:],
                        op=mybir.AluOpType.is_equal,
                    )
                    nc.tensor.matmul(pool_ps[:], st[:], sel[:],
                                     start=(t == 0), stop=(t == NT - 1))

                if (not last) and t in group_last:
                    g = int(np.searchsorted(group_last, t))
                    s_row = int(GROUP_OFF[g])
                    l_row = int(GROUP_ROWS[g])
                    nc.gpsimd.collective_compute(
                        "AllGather", mybir.AluOpType.bypass,
                        ins=[newx[s_row:s_row + l_row, :]],
                        outs=[dst_rep[N_CORES * s_row:
                                      N_CORES * s_row + N_CORES * l_row, :]],
                        replica_groups=rg)

        # ---- mean pool + classifier ----
        pacc = wpool.tile([128, NUM_GRAPHS], f32, tag="pacc")
        nc.vector.tensor_tensor(out=pacc[:], in0=pool_ps[:], in1=inv_t[:],
                                op=mybir.AluOpType.mult)
        nc.sync.dma_start(pr_in[:], pacc[:])
        nc.gpsimd.collective_compute(
            "AllReduce", mybir.AluOpType.add,
            ins=[pr_in[:]], outs=[pr_out[:]], replica_groups=rg)
        pooled = wpool.tile([128, NUM_GRAPHS], f32, tag="pooled")
        nc.sync.dma_start(pooled[:], pr_out[:])

        pc1 = psum.tile([F, NUM_GRAPHS], f32, tag="aggr", bufs=2)
        nc.tensor.matmul(pc1[:], wc1_t[:], pooled[:], start=True, stop=True)
        rc1 = wpool.tile([F, NUM_GRAPHS], f32, tag="rc1")
        nc.scalar.activation(rc1[:], pc1[:], mybir.ActivationFunctionType.Relu,
                             bias=bc1_t[:])
        pc2 = psum.tile([NUM_CLASSES, NUM_GRAPHS], f32, tag="p1", bufs=1)
        nc.tensor.matmul(pc2[:], wc2_t[:], rc1[:], start=True, stop=True)
        lg = wpool.tile([NUM_CLASSES, NUM_GRAPHS], f32, tag="lg")
        nc.vector.tensor_scalar(out=lg[:], in0=pc2[:], scalar1=bc2_t[:],
                                scalar2=None, op0=mybir.AluOpType.add)
        nc.sync.dma_start(out_t[:], lg[:])
        ctx.close()

    nc.compile()
    return nc


def _get_module(nch, nchb, colbase):
    key = tuple(nch.tolist()) + tuple(nchb.ravel().tolist())
    if key not in _CACHE:
        _CACHE.clear()
        _CACHE[key] = _build(nch, nchb, colbase)
    return _CACHE[key]


def kernel(x, edge_index, batch, eps, W1, b1, W2, b2, Wc1, bc1, Wc2, bc2,
           _trace=False):
    import ml_dtypes
    from concourse.bass_utils import run_bass_kernel_spmd

    x = np.ascontiguousarray(np.asarray(x), dtype=np.float32)
    eps = np.asarray(eps, dtype=np.float32)
    W1 = np.asarray(W1, dtype=np.float32)
    b1 = np.asarray(b1, dtype=np.float32)
    W2 = np.asarray(W2, dtype=np.float32)
    b2 = np.asarray(b2, dtype=np.float32)

    gidx, idx16, gdst, nch, nchb, colbase, blocal, inv = _preprocess(
        np.asarray(edge_index), np.asarray(batch))
    nc = _get_module(nch, nchb, colbase)

    # group-major bf16 replica of the initial x
    xrep0 = np.empty((N_NODES, F), dtype=ml_dtypes.bfloat16)
    x_by_core = x.reshape(N_CORES, NPC, F)
    for g in range(len(GROUP_TILES)):
        s, e = int(GROUP_OFF[g]), int(GROUP_OFF[g + 1])
        xrep0[N_CORES * s:N_CORES * e] = (
            x_by_core[:, s:e, :].reshape(-1, F).astype(ml_dtypes.bfloat16))

    L = NUM_LAYERS
    common = {
        "xrep0": xrep0,
        "iota": np.ascontiguousarray(np.broadcast_to(
            np.arange(128, dtype=np.float32), (128, 128))).astype(
                ml_dtypes.bfloat16),
        "iotaf": np.ascontiguousarray(np.broadcast_to(
            np.arange(NUM_GRAPHS, dtype=np.float32), (128, NUM_GRAPHS))),
        "ident": np.eye(128, dtype=np.float32),
        "w1cat": np.ascontiguousarray(
            np.concatenate(list(W1[:L]), axis=1)).astype(ml_dtypes.bfloat16),
        "w2cat": np.ascontiguousarray(
            np.concatenate(list(W2[:L]), axis=1)).astype(ml_dtypes.bfloat16),
        "b1t": np.ascontiguousarray(b1[:L].T),
        "b2t": np.ascontiguousarray(b2[:L].T),
        "epsb": np.ascontiguousarray(
            np.broadcast_to(1.0 + eps[:L], (F, L))),
        "wc1": np.ascontiguousarray(np.asarray(Wc1, np.float32)),
        "bc1c": np.ascontiguousarray(np.asarray(bc1, np.float32)[:, None]),
        "wc2": np.ascontiguousarray(np.asarray(Wc2, np.float32)),
        "bc2c": np.ascontiguousarray(np.asarray(bc2, np.float32)[:, None]),
        "invc": np.ascontiguousarray(np.broadcast_to(inv, (128, NUM_GRAPHS))),
    }
    in_maps = []
    for c in range(N_CORES):
        m = dict(common)
        m["xT_own"] = np.ascontiguousarray(x[c * NPC:(c + 1) * NPC].T)
        m["gidx"] = gidx[c]
        m["idx16"] = idx16[c]
        m["gdst"] = gdst[c].astype(ml_dtypes.bfloat16)
        m["blocal"] = blocal[c]
        in_maps.append(m)

    res = run_bass_kernel_spmd(nc, in_maps, core_ids=list(range(N_CORES)),
                               trace=_trace)
    out = np.ascontiguousarray(res.results[0]["logits_t"].T)
    if _trace:
        kernel._last_result = res
    return out
